# revision 8
# baseline (speedup 1.0000x reference)
"""Trainium2 Bass kernel for nn_Attention_90486370992549.

Learned-sigmoid-mask multi-head attention:
  qkv = x @ W_qkv.T + b_qkv
  attn = softmax((q k^T / sqrt(D)) * sigmoid(att_mask))
  out  = (attn @ v) @ W_proj.T + b_proj

Sharding: data-parallel over batch across 8 NeuronCores (16 batches/core).
All matmuls run in float32r (tf32-like PE mode, ~1e-4 relative rounding,
full 1 cycle/row rate when the moving free dim >= 256).

Per-core plan, processed in 8 chunks of 2 batches (392 tokens):
  - x^T via PE transpose (contraction must sit on partitions)
  - qk^T = (W_qk x^T) in outc-major layout -> per-head q,k are D-major
  - V in token-major layout (separate matmul, x^T as stationary)
  - per (batch, head): S^T = k^T q (free dim padded to 256), multiply by
    sigmoid-mask (pre-scaled, transposed, host-side), exp (no max-subtract:
    logits are ~N(0, 0.16)), PV with a ones-column in V giving the softmax
    denominator as row 64 of the PSUM output
  - reciprocal of the denominator row; broadcast across partitions via a
    DRAM round-trip DMA (engines cannot partition-broadcast)
  - proj uses O^T as the matmul stationary -> token-major output, no final
    transpose; proj of chunk k-1 is emitted inside chunk k so the in-order
    PE queue never stalls on the normalization barrier.
"""

import numpy as np

B, N, C, H, D = 128, 196, 768, 12, 64
SCALE = D ** -0.5
NCORES = 8
BPC = B // NCORES              # batches per core
BPCHUNK = 2                    # batches per chunk
NCHUNK = BPC // BPCHUNK        # 8 chunks
T = BPCHUNK * N                # 392 tokens per chunk
TOK_TILES = [(0, 128), (128, 128), (256, 128), (384, 8)]
MC = [(0, 128), (128, 68)]     # m-chunks within one batch (196 = 128 + 68)
QP = 456                       # qk^T buffer width (392 + 64 pad for q windows)
SPAD = 256                     # padded free dim for attention matmuls

_CACHE = {}


def _build():
    from contextlib import ExitStack

    import concourse.bacc as bacc
    import concourse.bass as bass
    import concourse.mybir as mybir
    from concourse.masks import make_identity
    from concourse.tile import TileContext

    f32 = mybir.dt.float32
    f32r = mybir.dt.float32r
    AF = mybir.ActivationFunctionType
    OP = mybir.AluOpType

    nc = bacc.Bacc("TRN2", target_bir_lowering=False, debug=False,
                   num_devices=NCORES)
    x = nc.dram_tensor("x", [BPC * N, C], f32r, kind="ExternalInput")
    wqkT = nc.dram_tensor("wqkT", [C, 2 * C], f32r, kind="ExternalInput")
    wvT = nc.dram_tensor("wvT", [C, C], f32r, kind="ExternalInput")
    wpT = nc.dram_tensor("wpT", [C, C], f32r, kind="ExternalInput")
    bqk = nc.dram_tensor("bqk", [128, 12], f32, kind="ExternalInput")
    bv = nc.dram_tensor("bv", [1, C], f32, kind="ExternalInput")
    bp = nc.dram_tensor("bp", [1, C], f32, kind="ExternalInput")
    maskA = nc.dram_tensor("maskA", [128, H, N], f32, kind="ExternalInput")
    maskB = nc.dram_tensor("maskB", [68, H, N], f32, kind="ExternalInput")
    y = nc.dram_tensor("y", [BPC * N, C], f32, kind="ExternalOutput")

    with TileContext(nc) as tc, ExitStack() as ctx:
        singles = ctx.enter_context(tc.tile_pool(name="singles", bufs=1))
        xnat_p = ctx.enter_context(tc.tile_pool(name="xnat", bufs=2))
        xT_p = ctx.enter_context(tc.tile_pool(name="xT", bufs=2))
        qkT_p = ctx.enter_context(tc.tile_pool(name="qkT", bufs=1))
        v_p = ctx.enter_context(tc.tile_pool(name="v", bufs=4))
        ot_p = ctx.enter_context(tc.tile_pool(name="ot", bufs=2))
        p_p = ctx.enter_context(tc.tile_pool(name="p", bufs=6))
        y_p = ctx.enter_context(tc.tile_pool(name="y", bufs=2))
        rc_p = ctx.enter_context(tc.tile_pool(name="rc", bufs=4))
        bc_p = ctx.enter_context(tc.tile_pool(name="bc", bufs=1))
        dram_p = ctx.enter_context(tc.tile_pool(name="dram", bufs=2,
                                                space="DRAM"))
        ps_misc = ctx.enter_context(tc.tile_pool(name="psm", bufs=2,
                                                 space="PSUM"))
        ps_s = ctx.enter_context(tc.tile_pool(name="pss", bufs=2,
                                              space="PSUM"))
        ps_o = ctx.enter_context(tc.tile_pool(name="pso", bufs=2,
                                              space="PSUM"))
        ps_vp = ctx.enter_context(tc.tile_pool(name="psvp", bufs=2,
                                               space="PSUM"))

        # --- resident weights / constants ---
        wqk_sb = singles.tile([128, 6, 2 * C], f32r)
        nc.sync.dma_start(wqk_sb[:], wqkT.rearrange("(ko p) n -> p ko n", p=128))
        wv_sb = singles.tile([128, 6, C], f32r)
        nc.sync.dma_start(wv_sb[:], wvT.rearrange("(ko p) n -> p ko n", p=128))
        wp_sb = singles.tile([128, 6, C], f32r)
        nc.sync.dma_start(wp_sb[:], wpT.rearrange("(ko p) n -> p ko n", p=128))
        bqk_sb = singles.tile([128, 12], f32)
        nc.sync.dma_start(bqk_sb[:], bqk[:])
        bv_sb = singles.tile([128, C], f32)
        bv_ap = bv.ap()
        nc.sync.dma_start(bv_sb[:], bass.AP(
            tensor=bv_ap.tensor, offset=bv_ap.offset,
            ap=[[0, 128], bv_ap.ap[1]]))
        bp_sb = singles.tile([128, C], f32)
        bp_ap = bp.ap()
        nc.sync.dma_start(bp_sb[:], bass.AP(
            tensor=bp_ap.tensor, offset=bp_ap.offset,
            ap=[[0, 128], bp_ap.ap[1]]))
        mA_sb = singles.tile([128, H, N], f32)
        nc.sync.dma_start(mA_sb[:], maskA[:])
        mB_sb = singles.tile([68, H, N], f32)
        nc.sync.dma_start(mB_sb[:], maskB[:])
        ident_f = singles.tile([128, 128], f32)
        make_identity(nc, ident_f[:])
        ident = singles.tile([128, 128], f32r)
        nc.vector.tensor_copy(ident[:], ident_f[:])

        def emit_proj(ot, ck):
            """proj: y[tok, outc] = O^T.T @ wpT (+ bp), then DMA out."""
            for (off, rows) in TOK_TILES:
                ph = [ps_vp.tile([128, 384], f32, tag="vp", name="ph")[:rows]
                      for _ in range(2)]
                for j in range(6):
                    lhs = ot[:, j, off:off + rows]
                    for half in range(2):
                        nc.tensor.matmul(
                            ph[half], lhs,
                            wp_sb[:, j, half * 384:(half + 1) * 384],
                            start=(j == 0), stop=(j == 5))
                y_sb = y_p.tile([128, C], f32, tag="y")
                for half in range(2):
                    nc.vector.tensor_tensor(
                        y_sb[:rows, half * 384:(half + 1) * 384],
                        ph[half], bp_sb[:rows, half * 384:(half + 1) * 384],
                        OP.add)
                nc.sync.dma_start(
                    y[ck * T + off: ck * T + off + rows, :], y_sb[:rows])

        prev = None
        for ck in range(NCHUNK):
            # --- load x, build x^T via PE transpose ---
            xT = xT_p.tile([128, 6, T], f32r, tag="xT")
            for (off, rows) in TOK_TILES:
                xn = xnat_p.tile([128, C], f32r, tag="xn")
                nc.sync.dma_start(
                    xn[:rows], x[ck * T + off: ck * T + off + rows, :])
                for j in range(6):
                    pst = ps_misc.tile([128, 392], f32r, tag="misc")
                    nc.tensor.transpose(
                        pst[:, :rows], xn[:rows, j * 128:(j + 1) * 128],
                        ident[:rows, :rows])
                    nc.scalar.activation(
                        xT[:, j, off:off + rows], pst[:, :rows], AF.Copy)

            # --- qk^T = W_qk @ x^T  [12 tiles of 128 outc, T tokens] ---
            qkT = qkT_p.tile([128, 12, QP], f32r, tag="qkT")
            nc.gpsimd.memset(qkT[:, :, T:QP].bitcast(f32), 0.0)
            for i in range(12):
                pq = ps_misc.tile([128, 392], f32, tag="misc")
                for j in range(6):
                    nc.tensor.matmul(
                        pq[:], wqk_sb[:, j, i * 128:(i + 1) * 128],
                        xT[:, j, :], start=(j == 0), stop=(j == 5))
                nc.vector.tensor_scalar_add(
                    qkT[:, i, :T], pq[:], bqk_sb[:, i:i + 1])

            # software-pipelined proj of the previous chunk
            if prev is not None:
                emit_proj(*prev)

            # --- V token-major, per batch-m-chunk slices ---
            vts = []
            for b in range(BPCHUNK):
                for (moff, mrows) in MC:
                    soff = b * N + moff
                    vt = v_p.tile([128, H, D + 1], f32r, tag="v")
                    pv = [ps_vp.tile([128, 384], f32, tag="vp", name="pv")[:mrows]
                          for _ in range(2)]
                    for j in range(6):
                        lhs = xT[:, j, soff:soff + mrows]
                        for half in range(2):
                            nc.tensor.matmul(
                                pv[half], lhs,
                                wv_sb[:, j, half * 384:(half + 1) * 384],
                                start=(j == 0), stop=(j == 5))
                    for half in range(2):
                        nc.vector.tensor_tensor(
                            vt[:mrows, half * 6:(half + 1) * 6, :D],
                            pv[half].rearrange("p (h d) -> p h d", d=D),
                            bv_sb[:mrows, half * 384:(half + 1) * 384]
                            .rearrange("p (h d) -> p h d", d=D),
                            OP.add)
                    nc.gpsimd.memset(vt[:mrows, :, D:D + 1].bitcast(f32), 1.0)
                    vts.append(vt)

            # --- attention per (batch, head) ---
            ot = ot_p.tile([128, 6, T], f32r, tag="ot")
            scr = dram_p.tile([24, N], f32, name="scr")
            for b in range(BPCHUNK):
                for h in range(H):
                    hp, j = h % 2, h // 2
                    q_ap = qkT[hp * 64:(hp + 1) * 64, j,
                               b * N: b * N + SPAD]
                    ps = ps_s.tile([128, 2, SPAD], f32, tag="s")
                    ptiles = []
                    for mi, (moff, mrows) in enumerate(MC):
                        k_ap = qkT[hp * 64:(hp + 1) * 64, 6 + j,
                                   b * N + moff: b * N + moff + mrows]
                        nc.tensor.matmul(ps[:mrows, mi, :], k_ap, q_ap,
                                         start=True, stop=True)
                        pt = p_p.tile([128, SPAD], f32r, tag="p")
                        m_sb = (mA_sb if mi == 0 else mB_sb)
                        nc.gpsimd.memset(pt[:mrows, N:SPAD].bitcast(f32), 0.0)
                        nc.vector.tensor_tensor(
                            pt[:mrows, :N], ps[:mrows, mi, :N],
                            m_sb[:mrows, h, :], OP.mult)
                        nc.scalar.activation(pt[:mrows, :N], pt[:mrows, :N],
                                             AF.Exp)
                        ptiles.append(pt)
                    po = ps_o.tile([D + 1, SPAD], f32, tag="o")
                    for mi, (moff, mrows) in enumerate(MC):
                        nc.tensor.matmul(
                            po[:], vts[b * 2 + mi][:mrows, h, :],
                            ptiles[mi][:mrows, :],
                            start=(mi == 0), stop=(mi == 1))
                    rt = rc_p.tile([1, N], f32, tag="rc")
                    nc.vector.reciprocal(rt[:], po[D:D + 1, :N])
                    nc.sync.dma_start(scr[2 * h + b: 2 * h + b + 1, :], rt[:])
                    nc.scalar.activation(
                        ot[hp * 64:(hp + 1) * 64, j, b * N:(b + 1) * N]
                        .bitcast(f32r),
                        po[:D, :N], AF.Copy)

            # --- softmax denominators: broadcast via DRAM round-trip ---
            bc = bc_p.tile([128, H, N], f32, tag="bc")
            scr_ap = scr[:]
            for hp in range(2):
                nc.sync.dma_start(
                    bc[hp * 64:(hp + 1) * 64].rearrange(
                        "p (a b) n -> p a b n", b=2),
                    bass.AP(tensor=scr_ap.tensor,
                            offset=scr_ap.offset + hp * 2 * N,
                            ap=[[0, 64], [4 * N, 6], [N, 2], [1, N]]))
            for j in range(6):
                for b in range(BPCHUNK):
                    sl = ot[:, j, b * N:(b + 1) * N]
                    nc.vector.tensor_tensor(
                        sl.bitcast(f32r), sl.bitcast(f32), bc[:, 2 * j + b, :],
                        OP.mult)

            prev = (ot, ck)
        emit_proj(*prev)

    nc.compile()
    return nc


def _get_nc():
    if "nc" not in _CACHE:
        _CACHE["nc"] = _build()
    return _CACHE["nc"]


def _prep_shared(W_qkv, b_qkv, att_mask, W_proj, b_proj):
    W_qkv = np.asarray(W_qkv, np.float32)
    W_proj = np.asarray(W_proj, np.float32)
    b_qkv = np.asarray(b_qkv, np.float32)
    b_proj = np.asarray(b_proj, np.float32)
    att_mask = np.asarray(att_mask, np.float32)
    sig = SCALE / (1.0 + np.exp(-att_mask))          # [H, n, m]
    maskT = np.ascontiguousarray(sig.transpose(0, 2, 1))  # [H, m, n]
    return {
        "wqkT": np.ascontiguousarray(W_qkv[:2 * C].T),
        "wvT": np.ascontiguousarray(W_qkv[2 * C:].T),
        "wpT": np.ascontiguousarray(W_proj.T),
        "bqk": np.ascontiguousarray(b_qkv[:2 * C].reshape(12, 128).T),
        "bv": np.ascontiguousarray(b_qkv[2 * C:].reshape(1, C)),
        "bp": np.ascontiguousarray(b_proj.reshape(1, C)),
        "maskA": np.ascontiguousarray(maskT[:, :128, :].transpose(1, 0, 2)),
        "maskB": np.ascontiguousarray(maskT[:, 128:, :].transpose(1, 0, 2)),
    }


def kernel(x, W_qkv, b_qkv, att_mask, W_proj, b_proj):
    from concourse.bass_utils import run_bass_kernel_spmd

    x = np.asarray(x, np.float32)
    nc = _get_nc()
    shared = _prep_shared(W_qkv, b_qkv, att_mask, W_proj, b_proj)
    in_maps = []
    for c in range(NCORES):
        m = dict(shared)
        m["x"] = np.ascontiguousarray(
            x[c * BPC:(c + 1) * BPC].reshape(BPC * N, C))
        in_maps.append(m)
    res = run_bass_kernel_spmd(nc, in_maps, core_ids=list(range(NCORES)))
    out = np.stack([res.results[c]["y"].reshape(BPC, N, C)
                    for c in range(NCORES)])
    return out.reshape(B, N, C).astype(np.float32)
